# revision 13
# baseline (speedup 1.0000x reference)
"""Trainium2 Bass kernel for a post-LN transformer block (MHA + FFN).

Contract: kernel(**inputs) takes FULL unsharded inputs, returns the FULL
output [2, 2048, 1024].

v4: head-sharded attention (core c owns heads {2c,2c+1}, both batches),
batch-1 QKV matmuls drained at key-block granularity inside batch-0
attention units, V built via V^T + PE transpose, one 8-rank mesh AllToAll
of ctx, post phases feature-major with residuals/biases folded into
identity/diagonal matmuls and LN gamma/beta folded into fc1 weights.
Softmax exp runs on ACT; batch-1 units offload 4/16 key blocks to a
DVE Schraudolph exp2 approximation.
"""
import sys

for _p in ('/opt/trn_rl_repo', '/opt/pypackages'):
    if _p not in sys.path:
        sys.path.insert(0, _p)

import numpy as np
import ml_dtypes
import concourse.bass as bass
import concourse.tile as tile
from concourse import bacc, mybir
from concourse.bass import ts
from concourse.masks import make_identity
from contextlib import ExitStack

def _install_prof_shim():
    import types
    if 'antenv.axon_hooks' in sys.modules:
        return
    try:
        import trn_agent_boot.trn_boot as tb
        hook = tb._ntff_profile_via_ctypes('/opt/axon/libaxon_pjrt.so')
    except Exception:
        hook = None
    mod = types.ModuleType('antenv.axon_hooks')
    mod.get_axon_ntff_profile_hook = lambda: hook
    mod.set_axon_ntff_profile_hook = lambda h: None
    sys.modules['antenv.axon_hooks'] = mod

_install_prof_shim()

from concourse.bass_utils import run_bass_kernel_spmd  # noqa: E402

B, S, H, NH, HD = 2, 2048, 1024, 16, 64
P = 128
NCORES = 8
TQ = S // 4
FT = H // P
KB = S // P
QB = S // TQ
EPS = 1e-5
RG8 = [[0, 1, 2, 3, 4, 5, 6, 7]]
MAGIC = 0x5f3759df + 1
SCH_A = 12102203.161561485      # 2^23 / ln 2
SCH_B = 1064866805.0            # 127*2^23 - 486411 (RMS-optimal Schraudolph)
DVE_KB = (3, 7, 11, 15)         # key blocks exp'd on DVE in batch-1 units

f32 = mybir.dt.float32
bf16 = mybir.dt.bfloat16
i32 = mybir.dt.int32
AF = mybir.ActivationFunctionType
ALU = mybir.AluOpType

W_CHUNK = 256


def build_kernel():
    nc = bacc.Bacc("TRN2", target_bir_lowering=False, debug=False,
                   num_devices=NCORES)

    def din(name, shape, dt=f32):
        return nc.dram_tensor(name, shape, dt, kind="ExternalInput").ap()

    xT = din("xT", [H, B * S], bf16)          # x^T, both batches
    xresb = din("xresb", [P, FT, TQ], bf16)   # my tokens x^T (pretiled, bf16)
    wq4 = din("wq4", [P, FT, P], bf16)        # pretiled Wq.T slice
    wk4 = din("wk4", [P, FT, P], bf16)        # pretiled, * 1/8
    wv4 = din("wv4", [P, FT, P], bf16)
    wo4 = din("wo4", [4, P, FT, W_CHUNK], bf16)   # pre-chunked Wo.T
    w14 = din("w14", [4, P, FT, W_CHUNK], bf16)   # pre-chunked (W1*g1).T
    w24 = din("w24", [4, P, FT, W_CHUNK], bf16)   # pre-chunked W2.T
    g1d = din("g1d", [P, FT, P], bf16)        # diag(g1) per block
    b2d = din("b2d", [P, FT, P], bf16)        # diag(b2 + be1) per block
    bqp = din("bqp", [P, 1])
    bkp = din("bkp", [P, 1])
    bvp = din("bvp", [P, 1])
    b1p = din("b1p", [P, FT])                 # b1 + W1 @ be1
    g2p = din("g2p", [P, FT])
    be2p = din("be2p", [P, FT])
    nri = din("nri", [1, 2], i32)
    nrm = din("nrm", [1, TQ], i32)
    y = nc.dram_tensor("y", [H, TQ], f32, kind="ExternalOutput").ap()

    a2a_in = nc.dram_tensor("a2a_in", [NCORES, P, TQ], bf16).ap()
    a2a_out = nc.dram_tensor("a2a_out", [NCORES, P, TQ], bf16).ap()

    with tile.TileContext(nc) as tc, ExitStack() as ctx:
        const = ctx.enter_context(tc.tile_pool(name="const", bufs=1))
        acts = ctx.enter_context(tc.tile_pool(name="acts", bufs=1))
        wpool = ctx.enter_context(tc.tile_pool(name="w", bufs=3))

        # ---- early DMAs: weights for b0 QKV, then x tiles (b0 first) ----
        xpctx = ExitStack()
        xp = xpctx.enter_context(tc.tile_pool(name="xp", bufs=1))
        wq_s = xp.tile([P, FT, P], bf16)
        nc.sync.dma_start(wq_s[:], wq4)
        wk_s = xp.tile([P, FT, P], bf16)
        nc.sync.dma_start(wk_s[:], wk4)
        xt_r = xT.rearrange("(t p) (b n) -> p b t n", p=P, b=B)
        xt = [[None] * FT for _ in range(B)]
        for b in range(B):
            for kt in range(FT):
                xt[b][kt] = xp.tile([P, S], bf16, tag=f"xt{b}_{kt}",
                                    name=f"xt{b}_{kt}")
        for kt in range(FT):
            nc.sync.dma_start(xt[0][kt][:], xt_r[:, 0, kt, :])
        wv_s = xp.tile([P, FT, P], bf16)
        nc.sync.dma_start(wv_s[:], wv4)
        for kt in range(FT):
            nc.sync.dma_start(xt[1][kt][:], xt_r[:, 1, kt, :])

        # constants (gpsimd queue so they don't delay x)
        bq_s = const.tile([P, 1], f32)
        nc.gpsimd.dma_start(bq_s[:], bqp)
        bk_s = const.tile([P, 1], f32)
        nc.gpsimd.dma_start(bk_s[:], bkp)
        bv_s = const.tile([P, 1], f32)
        nc.gpsimd.dma_start(bv_s[:], bvp)
        b1_s = const.tile([P, FT], f32)
        nc.gpsimd.dma_start(b1_s[:], b1p)
        g2_s = const.tile([P, FT], f32)
        nc.gpsimd.dma_start(g2_s[:], g2p)
        be2_s = const.tile([P, FT], f32)
        nc.gpsimd.dma_start(be2_s[:], be2p)
        nri_s = const.tile([1, 2], i32)
        nc.gpsimd.dma_start(nri_s[:], nri)
        nrm_s = const.tile([1, TQ], i32)
        nc.gpsimd.dma_start(nrm_s[:], nrm)
        g1d_s = const.tile([P, FT, P], bf16)
        nc.gpsimd.dma_start(g1d_s[:], g1d)
        b2d_s = const.tile([P, FT, P], bf16)
        nc.gpsimd.dma_start(b2d_s[:], b2d)
        xres_s = const.tile([P, FT, TQ], bf16)
        nc.gpsimd.dma_start(xres_s[:], xresb)
        ones_b = const.tile([P, 1], bf16)
        nc.vector.memset(ones_b[:], 1.0)
        ones_w = const.tile([P, TQ], bf16)
        nc.vector.memset(ones_w[:], 1.0)
        identb = const.tile([P, P], bf16)
        make_identity(nc, identb)
        eps_s = const.tile([P, 1], f32)
        nc.vector.memset(eps_s[:], EPS)
        warm_s = const.tile([P, 1], f32)
        nc.scalar.activation(warm_s[:], eps_s[:], AF.Exp)

        # persistent activations
        qt_s = acts.tile([P, B, S], bf16)
        kt_s = acts.tile([P, B, S], bf16)
        v_s = acts.tile([P, B, KB, 2, HD + 1], bf16)
        ctxF_s = acts.tile([P, FT, TQ], bf16)
        ln1b_s = acts.tile([P, FT, TQ], bf16)   # LN1 "z" (gamma folded into W1)
        hT_s = acts.tile([P, FT, TQ], bf16)
        for b in range(B):
            nc.vector.memset(v_s[:, b, :, :, HD:HD + 1], 1.0)
        vtctx = ExitStack()
        vtp = vtctx.enter_context(tc.tile_pool(name="vtp", bufs=1))
        vt0 = vtp.tile([P, S], bf16)            # V^T staging per batch
        vt1 = vtp.tile([P, S], bf16)

        # ---------------- batch-0 Q/K (kt-outer for early start) ----------
        with tc.tile_pool(name="psQK", bufs=1, space="PSUM") as psQK:
            pq = psQK.tile([P, QB, TQ], f32, tag="pq")
            pk = psQK.tile([P, QB, TQ], f32, tag="pk")
            for kt in range(FT):
                for c in range(QB):
                    nc.tensor.matmul(pq[:, c, :], wq_s[:, kt, :],
                                     xt[0][kt][:, ts(c, TQ)],
                                     start=(kt == 0), stop=(kt == FT - 1))
                for c in range(QB):
                    nc.tensor.matmul(pk[:, c, :], wk_s[:, kt, :],
                                     xt[0][kt][:, ts(c, TQ)],
                                     start=(kt == 0), stop=(kt == FT - 1))
            for c in range(QB):
                nc.scalar.activation(qt_s[:, 0, ts(c, TQ)], pq[:, c, :],
                                     AF.Identity, bias=bq_s[:, 0:1])
                nc.scalar.activation(kt_s[:, 0, ts(c, TQ)], pk[:, c, :],
                                     AF.Identity, bias=bk_s[:, 0:1])

        # batch-0 V: V^T chunks then PE transposes into v_s
        with tc.tile_pool(name="psV0", bufs=2, space="PSUM") as psV0, \
             tc.tile_pool(name="psT0", bufs=2, space="PSUM") as psT0:
            for c in range(QB):
                ps = psV0.tile([P, TQ], f32, tag="pv")
                for kt in range(FT):
                    nc.tensor.matmul(ps[:], wv_s[:, kt, :],
                                     xt[0][kt][:, ts(c, TQ)],
                                     start=(kt == 0), stop=(kt == FT - 1))
                nc.scalar.activation(vt0[:, ts(c, TQ)], ps[:],
                                     AF.Identity, bias=bv_s[:, 0:1])
            for kb in range(KB):
                pt = psT0.tile([P, P], f32, tag="pt")
                nc.tensor.matmul(pt[:], vt0[:, ts(kb, P)], identb[:],
                                 start=True, stop=True)
                nc.vector.tensor_copy(
                    v_s[:, 0, kb, :, 0:HD],
                    pt.rearrange("p (h d) -> p h d", h=2))

        # ---------- batch-1 QKV (serial, before attention) ----------
        with tc.tile_pool(name="psB1", bufs=2, space="PSUM") as psB1, \
             tc.tile_pool(name="psT1", bufs=2, space="PSUM") as psT1:
            for w_s, dst, bias in [(wq_s, qt_s, bq_s), (wk_s, kt_s, bk_s)]:
                for c in range(QB):
                    ps = psB1.tile([P, TQ], f32, tag="pb1", name="pb1")
                    for kt in range(FT):
                        nc.tensor.matmul(ps[:], w_s[:, kt, :],
                                         xt[1][kt][:, ts(c, TQ)],
                                         start=(kt == 0), stop=(kt == FT - 1))
                    nc.scalar.activation(dst[:, 1, ts(c, TQ)], ps[:],
                                         AF.Identity, bias=bias[:, 0:1])
            for c in range(QB):
                ps = psB1.tile([P, TQ], f32, tag="pb1", name="pb1")
                for kt in range(FT):
                    nc.tensor.matmul(ps[:], wv_s[:, kt, :],
                                     xt[1][kt][:, ts(c, TQ)],
                                     start=(kt == 0), stop=(kt == FT - 1))
                nc.scalar.activation(vt1[:, ts(c, TQ)], ps[:],
                                     AF.Identity, bias=bv_s[:, 0:1])
            for kb in range(KB):
                pt = psT1.tile([P, P], f32, tag="pt1", name="pt1")
                nc.tensor.matmul(pt[:], vt1[:, ts(kb, P)], identb[:],
                                 start=True, stop=True)
                nc.vector.tensor_copy(
                    v_s[:, 1, kb, :, 0:HD],
                    pt.rearrange("p (h d) -> p h d", h=2))

        vtctx.close()

        # ---------------- attention ----------------
        with tc.tile_pool(name="psS", bufs=2, space="PSUM") as psS, \
             tc.tile_pool(name="psC", bufs=2, space="PSUM") as psC, \
             tc.tile_pool(name="esb", bufs=2) as esb, \
             tc.tile_pool(name="rec", bufs=2) as rec:

            def attn_unit(b, qb, do_drain):
                ps_c0 = psC.tile([P, TQ], f32, tag="c0")
                ps_c1 = psC.tile([P, TQ], f32, tag="c1")
                for kb in range(KB):
                    last = (kb == KB - 1)
                    ps = psS.tile([P, 2, TQ], f32, tag="s")
                    nc.tensor.matmul(ps[:, 0, :], kt_s[0:HD, b, ts(kb, P)],
                                     qt_s[0:HD, b, ts(qb, TQ)],
                                     start=True, stop=True)
                    nc.tensor.matmul(ps[:, 1, :], kt_s[HD:P, b, ts(kb, P)],
                                     qt_s[HD:P, b, ts(qb, TQ)],
                                     start=True, stop=True)
                    e = esb.tile([P, 2, TQ], bf16, tag="e")
                    if kb in DVE_KB:
                        # Schraudolph exp on DVE: i32(A*x+B) bits read as f32
                        fv = esb.tile([P, 2, TQ], f32, tag="fv", bufs=1)
                        nc.vector.tensor_scalar(
                            out=fv.bitcast(i32)[:], in0=ps[:], scalar1=SCH_A,
                            scalar2=SCH_B, op0=ALU.mult, op1=ALU.add)
                        nc.vector.tensor_copy(e[:], fv[:])
                    else:
                        nc.scalar.activation(e[:], ps[:], AF.Exp)
                    nc.tensor.matmul(ps_c0[0:HD + 1, :], v_s[:, b, kb, 0, :],
                                     e[:, 0, :], start=(kb == 0), stop=last)
                    nc.tensor.matmul(ps_c1[0:HD + 1, :], v_s[:, b, kb, 1, :],
                                     e[:, 1, :], start=(kb == 0), stop=last)
                # normalize rows 0-63 by row 64; ship straight to a2a_in
                j = b * QB + qb
                for h, psc in ((0, ps_c0), (1, ps_c1)):
                    sr = rec.tile([HD + 1, TQ], f32, tag=f"sr{h}",
                                  name=f"sr{h}")
                    nc.vector.tensor_copy(sr[HD:HD + 1, :], psc[HD:HD + 1, :])
                    rr = rec.tile([1, TQ], f32, tag=f"rr{h}", name=f"rr{h}")
                    nc.gpsimd.dma_start(rr[:], sr[HD:HD + 1, :])
                    nc.vector.reciprocal_approx_fast(rr[:], rr[:])
                    rb = rec.tile([HD, TQ], f32, tag=f"rb{h}", name=f"rb{h}")
                    nc.gpsimd.partition_broadcast(rb[:], rr[:])
                    ct = rec.tile([HD, TQ], bf16, tag=f"ct{h}", name=f"ct{h}")
                    nc.vector.tensor_tensor(out=ct[:], in0=psc[0:HD, :],
                                            in1=rb[:], op=ALU.mult)
                    nc.sync.dma_start(a2a_in[j, ts(h, HD), :], ct[:])

            for qb in range(QB):
                attn_unit(0, qb, False)
            for qb in range(QB):
                attn_unit(1, qb, False)
        xpctx.close()

        # ---------------- AllToAll ----------------
        w_pre = wpool.tile([P, FT, W_CHUNK], bf16, tag="w")
        nc.sync.dma_start(w_pre[:], wo4[0])
        nc.gpsimd.collective_compute(
            "AllToAll", ALU.bypass, replica_groups=RG8,
            ins=[a2a_in], outs=[a2a_out])
        nc.sync.dma_start(ctxF_s[:], a2a_out.rearrange("r p n -> p r n"))

        # ---------------- projection helper (feature-major out) -----------
        def proj_T(w4ap, kxmT_s, evict, psA, w_first=None, extra_mm=None):
            for half in range(H // W_CHUNK):
                if half == 0 and w_first is not None:
                    w_s = w_first
                else:
                    w_s = wpool.tile([P, FT, W_CHUNK], bf16, tag="w",
                                     name="w_s")
                    nc.sync.dma_start(w_s[:], w4ap[half])
                for mi in range(W_CHUNK // P):
                    mt_i = half * (W_CHUNK // P) + mi
                    ps = psA.tile([P, TQ], f32, tag="psA", name="psA")
                    for kt in range(FT):
                        nc.tensor.matmul(ps[:], w_s[:, kt, ts(mi, P)],
                                         kxmT_s[:, kt, :],
                                         start=(kt == 0),
                                         stop=(kt == FT - 1 and
                                               extra_mm is None))
                    if extra_mm is not None:
                        extra_mm(mt_i, ps)
                    evict(mt_i, ps)

        # ------------- stats accumulators (delayed emission) --------------
        def make_stats(lnp, psSt, name):
            st0 = psSt.tile([1, TQ], f32, tag=f"{name}0", name=f"{name}0")
            st1 = psSt.tile([1, TQ], f32, tag=f"{name}1", name=f"{name}1")
            pend = []

            def emit_one():
                mt_j, tbj, sqj = pend.pop(0)
                nc.tensor.matmul(st0[:], ones_b[:], tbj,
                                 start=(mt_j == 0), stop=(mt_j == FT - 1))
                nc.tensor.matmul(st1[:], ones_b[:], sqj[:],
                                 start=(mt_j == 0), stop=(mt_j == FT - 1))

            def feed(mt_i, tb):
                sq = lnp.tile([P, TQ], bf16, tag=f"sq{mt_i % 3}",
                              name="sq")
                nc.vector.tensor_tensor(out=sq[:], in0=tb, in1=tb,
                                        op=ALU.mult)
                pend.append((mt_i, tb, sq))
                if len(pend) > 1:
                    emit_one()

            def flush():
                while pend:
                    emit_one()

            return st0, st1, feed, flush

        # LN scalar chain: mu/rstd from st0/st1, broadcast to [P, TQ]
        def ln_chain(st0, st1, lnp):
            mu = lnp.tile([1, TQ], f32, tag="mu", name="mu")
            nc.vector.tensor_scalar(out=mu[:], in0=st0[:], scalar1=1.0 / H,
                                    scalar2=None, op0=ALU.mult)
            muB = lnp.tile([P, TQ], f32, tag="muB", name="muB")
            nc.gpsimd.partition_broadcast(muB[:], mu[:])
            ve = lnp.tile([1, TQ], f32, tag="ve", name="ve")
            nc.vector.tensor_scalar(out=ve[:], in0=st1[:], scalar1=1.0 / H,
                                    scalar2=None, op0=ALU.mult)
            mu2 = lnp.tile([1, TQ], f32, tag="mu2", name="mu2")
            nc.vector.tensor_tensor(out=mu2[:], in0=mu[:], in1=mu[:],
                                    op=ALU.mult)
            nc.vector.tensor_tensor(out=ve[:], in0=ve[:], in1=mu2[:],
                                    op=ALU.subtract)
            nc.vector.tensor_scalar(out=ve[:], in0=ve[:], scalar1=EPS,
                                    scalar2=None, op0=ALU.add)
            it = lnp.tile([1, TQ], i32, tag="it", name="it")
            nc.vector.tensor_scalar(out=it[:], in0=ve.bitcast(i32),
                                    scalar1=nri_s[:, 0:1], scalar2=None,
                                    op0=ALU.logical_shift_right)
            nc.vector.tensor_scalar(out=it[:], in0=it[:],
                                    scalar1=nri_s[:, 1:2],
                                    scalar2=None, op0=ALU.bitwise_xor)
            nc.vector.tensor_tensor(out=it[:], in0=it[:], in1=nrm_s[:],
                                    op=ALU.add)
            rstd = it.bitcast(f32)
            nrt = lnp.tile([1, TQ], f32, tag="nrt", name="nrt")
            for _ in range(2):
                nc.vector.tensor_tensor(out=nrt[:], in0=rstd, in1=rstd,
                                        op=ALU.mult)
                nc.vector.tensor_tensor(out=nrt[:], in0=nrt[:], in1=ve[:],
                                        op=ALU.mult)
                nc.vector.tensor_scalar(out=nrt[:], in0=nrt[:], scalar1=-0.5,
                                        scalar2=1.5, op0=ALU.mult,
                                        op1=ALU.add)
                nc.vector.tensor_tensor(out=rstd, in0=rstd, in1=nrt[:],
                                        op=ALU.mult)
            rsB = lnp.tile([P, TQ], f32, tag="rsB", name="rsB")
            nc.gpsimd.partition_broadcast(rsB[:], rstd)
            return muB, rsB

        # ---------------- phases C-F ----------------
        with tc.tile_pool(name="lnp", bufs=2) as lnp, \
             tc.tile_pool(name="psA2", bufs=2, space="PSUM") as psA2, \
             tc.tile_pool(name="psSt", bufs=1, space="PSUM") as psSt:
            t1_s = acts.tile([P, FT, TQ], f32, tag="tres")
            t1b_s = acts.tile([P, FT, TQ], bf16, tag="tbb")
            st0a, st1a, feed_a, flush_a = make_stats(lnp, psSt, "sa")

            def wo_extra(mt_i, ps):    # += x residual via identity matmul
                nc.tensor.matmul(ps[:], identb[:], xres_s[:, mt_i, :],
                                 start=False, stop=True)

            def wo_evict(mt_i, ps):
                nc.scalar.copy(t1_s[:, mt_i, :], ps[:])
                nc.scalar.copy(t1b_s[:, mt_i, :], ps[:])
                feed_a(mt_i, t1b_s[:, mt_i, :])

            proj_T(wo4, ctxF_s, wo_evict, psA2, w_first=w_pre,
                   extra_mm=wo_extra)
            flush_a()
            muB1, rsB1 = ln_chain(st0a, st1a, lnp)
            for ft in range(FT):       # ln1b = (t1 - mu) * rstd  (bf16)
                z = lnp.tile([P, TQ], f32, tag=f"z{ft % 2}", name="z")
                nc.vector.tensor_tensor(out=z[:], in0=t1_s[:, ft, :],
                                        in1=muB1[:], op=ALU.subtract)
                nc.vector.tensor_tensor(out=ln1b_s[:, ft, :], in0=z[:],
                                        in1=rsB1[:], op=ALU.mult)

            def relu_evict(mt_i, ps):  # relu(ps + b1') on ACT
                nc.scalar.activation(hT_s[:, mt_i, :], ps[:], AF.Relu,
                                     bias=b1_s[:, mt_i:mt_i + 1])

            proj_T(w14, ln1b_s, relu_evict, psA2)

            t2_s = acts.tile([P, FT, TQ], f32, tag="tres")
            t2b_s = acts.tile([P, FT, TQ], bf16, tag="tbb")
            st0b, st1b, feed_b, flush_b = make_stats(lnp, psSt, "sb")

            def w2_extra(mt_i, ps):    # += g1*z (residual) and b2+be1
                nc.tensor.matmul(ps[:], g1d_s[:, mt_i, :],
                                 ln1b_s[:, mt_i, :], start=False, stop=False)
                nc.tensor.matmul(ps[:], b2d_s[:, mt_i, :], ones_w[:],
                                 start=False, stop=True)

            def w2_evict(mt_i, ps):
                nc.scalar.copy(t2_s[:, mt_i, :], ps[:])
                nc.scalar.copy(t2b_s[:, mt_i, :], ps[:])
                feed_b(mt_i, t2b_s[:, mt_i, :])

            proj_T(w24, hT_s, w2_evict, psA2, extra_mm=w2_extra)
            flush_b()
            muB2, rsB2 = ln_chain(st0b, st1b, lnp)
            y_r = y.rearrange("(t p) n -> p t n", p=P)
            for ft in range(FT):
                z = lnp.tile([P, TQ], f32, tag=f"z{ft % 2}", name="z")
                nc.vector.tensor_tensor(out=z[:], in0=t2_s[:, ft, :],
                                        in1=muB2[:], op=ALU.subtract)
                zg = lnp.tile([P, TQ], f32, tag=f"zg{ft % 2}", name="zg")
                nc.vector.scalar_tensor_tensor(
                    out=zg[:], in0=z[:], scalar=g2_s[:, ft:ft + 1],
                    in1=rsB2[:], op0=ALU.mult, op1=ALU.mult)
                yv = lnp.tile([P, TQ], f32, tag=f"yv{ft % 2}", name="yv")
                nc.scalar.activation(yv[:], zg[:], AF.Identity,
                                     bias=be2_s[:, ft:ft + 1])
                nc.sync.dma_start(y_r[:, ft, :], yv[:])

    nc.compile()
    return nc


_NC_CACHE = {}


def _get_nc():
    if 'nc' not in _NC_CACHE:
        _NC_CACHE['nc'] = build_kernel()
    return _NC_CACHE['nc']


def _bf(a):
    return np.ascontiguousarray(np.asarray(a, np.float32)).astype(
        ml_dtypes.bfloat16)


def _tile_w(wcolslice):
    # [H, 128] -> [P, FT, P] (kt-partition-tiled, contiguous)
    return np.ascontiguousarray(
        wcolslice.reshape(FT, P, P).transpose(1, 0, 2))


def _chunk_w(wT):
    # [H, H] -> [4, P, FT, W_CHUNK]
    w = wT.reshape(FT, P, 4, W_CHUNK)
    return np.ascontiguousarray(w.transpose(2, 1, 0, 3))


def make_in_maps(x, Wq, bq, Wk, bk, Wv, bv, Wo, bo, W1, b1, W2, b2,
                 g1, be1, g2, be2):
    def pt(v):
        return np.ascontiguousarray(np.asarray(v, np.float32).reshape(FT, P).T)

    def diagt(v):  # [H] -> [P, FT, P] block diagonals
        out = np.zeros((P, FT, P), np.float32)
        vv = np.asarray(v, np.float32).reshape(FT, P)
        for ftb in range(FT):
            np.fill_diagonal(out[:, ftb, :], 0)
            out[np.arange(P), ftb, np.arange(P)] = vv[ftb]
        return out

    scale = np.float32(1.0 / np.sqrt(HD))
    x = np.asarray(x, np.float32)
    g1 = np.asarray(g1, np.float32)
    be1 = np.asarray(be1, np.float32)
    W1 = np.asarray(W1, np.float32)
    b1 = np.asarray(b1, np.float32)
    b2 = np.asarray(b2, np.float32)
    bo = np.asarray(bo, np.float32)
    xTf = np.ascontiguousarray(x.transpose(2, 0, 1).reshape(H, B * S))
    wqT = np.asarray(Wq, np.float32).T
    wkT = np.asarray(Wk, np.float32).T * scale
    wvT = np.asarray(Wv, np.float32).T
    W1g = W1 * g1[None, :]
    b1f = b1 + W1 @ be1
    b2f = b2 + be1
    shared = {
        "xT": _bf(xTf),
        "wo4": _bf(_chunk_w(np.asarray(Wo, np.float32).T)),
        "w14": _bf(_chunk_w(W1g.T)),
        "w24": _bf(_chunk_w(np.asarray(W2, np.float32).T)),
        "g1d": _bf(diagt(g1)),
        "b2d": _bf(diagt(b2f)),
        "b1p": pt(b1f),
        "g2p": pt(g2), "be2p": pt(be2),
        "nri": np.array([[1, -1]], np.int32),
        "nrm": np.full((1, TQ), MAGIC, np.int32),
    }
    in_maps = []
    for c in range(NCORES):
        hs = slice(P * c, P * (c + 1))
        b, sl = c // 4, (c % 4) * TQ
        m = dict(shared)
        m["wq4"] = _bf(_tile_w(wqT[:, hs]))
        m["wk4"] = _bf(_tile_w(wkT[:, hs]))
        m["wv4"] = _bf(_tile_w(wvT[:, hs]))
        m["bqp"] = np.ascontiguousarray(
            np.asarray(bq, np.float32)[hs].reshape(P, 1))
        m["bkp"] = np.ascontiguousarray(
            (np.asarray(bk, np.float32) * scale)[hs].reshape(P, 1))
        m["bvp"] = np.ascontiguousarray(
            np.asarray(bv, np.float32)[hs].reshape(P, 1))
        xslice = (x[b, sl:sl + TQ, :].T + bo[:, None])  # [H, TQ] + bo
        m["xresb"] = _bf(xslice.reshape(FT, P, TQ).transpose(1, 0, 2))
        in_maps.append(m)
    return in_maps


def kernel(x, Wq, bq, Wk, bk, Wv, bv, Wo, bo, W1, b1, W2, b2,
           g1, be1, g2, be2):
    x = np.asarray(x)
    nc = _get_nc()
    in_maps = make_in_maps(x, Wq, bq, Wk, bk, Wv, bv, Wo, bo,
                           W1, b1, W2, b2, g1, be1, g2, be2)
    res = run_bass_kernel_spmd(nc, in_maps, list(range(NCORES)))
    out = np.empty((B, S, H), np.float32)
    for c in range(NCORES):
        b, sl = c // 4, (c % 4) * TQ
        out[b, sl:sl + TQ, :] = np.asarray(res.results[c]["y"]).T
    return out


# revision 15
# speedup vs baseline: 1.1833x; 1.1833x over previous
"""Trainium2 Bass kernel for a post-LN transformer block (MHA + FFN).

Contract: kernel(**inputs) takes FULL unsharded inputs, returns the FULL
output [2, 2048, 1024].

v4: head-sharded attention (core c owns heads {2c,2c+1}, both batches),
batch-1 QKV matmuls drained at key-block granularity inside batch-0
attention units, V built via V^T + PE transpose, one 8-rank mesh AllToAll
of ctx, post phases feature-major with residuals/biases folded into
identity/diagonal matmuls and LN gamma/beta folded into fc1 weights.
Softmax exp runs on ACT; batch-1 units offload 4/16 key blocks to a
DVE Schraudolph exp2 approximation.
"""
import sys

for _p in ('/opt/trn_rl_repo', '/opt/pypackages'):
    if _p not in sys.path:
        sys.path.insert(0, _p)

import numpy as np
import ml_dtypes
import concourse.bass as bass
import concourse.tile as tile
from concourse import bacc, mybir
from concourse.bass import ts
from concourse.masks import make_identity
from contextlib import ExitStack

def _install_prof_shim():
    import types
    if 'antenv.axon_hooks' in sys.modules:
        return
    try:
        import trn_agent_boot.trn_boot as tb
        hook = tb._ntff_profile_via_ctypes('/opt/axon/libaxon_pjrt.so')
    except Exception:
        hook = None
    mod = types.ModuleType('antenv.axon_hooks')
    mod.get_axon_ntff_profile_hook = lambda: hook
    mod.set_axon_ntff_profile_hook = lambda h: None
    sys.modules['antenv.axon_hooks'] = mod

_install_prof_shim()

from concourse.bass_utils import run_bass_kernel_spmd  # noqa: E402

B, S, H, NH, HD = 2, 2048, 1024, 16, 64
P = 128
NCORES = 8
TQ = S // 4
FT = H // P
KB = S // P
QB = S // TQ
EPS = 1e-5
RG8 = [[0, 1, 2, 3, 4, 5, 6, 7]]
MAGIC = 0x5f3759df + 1
SCH_A = 12102203.161561485      # 2^23 / ln 2
SCH_B = 1064866805.0            # 127*2^23 - 486411 (RMS-optimal Schraudolph)
DVE_KB = ()         # key blocks exp'd on DVE in batch-1 units

f32 = mybir.dt.float32
bf16 = mybir.dt.bfloat16
i32 = mybir.dt.int32
AF = mybir.ActivationFunctionType
ALU = mybir.AluOpType

W_CHUNK = 256


def build_kernel():
    nc = bacc.Bacc("TRN2", target_bir_lowering=False, debug=False,
                   num_devices=NCORES)

    def din(name, shape, dt=f32):
        return nc.dram_tensor(name, shape, dt, kind="ExternalInput").ap()

    xT = din("xT", [H, B * S], bf16)          # x^T, both batches
    xresb = din("xresb", [P, FT, TQ], bf16)   # my tokens x^T (pretiled, bf16)
    wq4 = din("wq4", [P, FT, P], bf16)        # pretiled Wq.T slice
    wk4 = din("wk4", [P, FT, P], bf16)        # pretiled, * 1/8
    wv4 = din("wv4", [P, FT, P], bf16)
    wo4 = din("wo4", [4, P, FT, W_CHUNK], bf16)   # pre-chunked Wo.T
    w14 = din("w14", [4, P, FT, W_CHUNK], bf16)   # pre-chunked (W1*g1).T
    w24 = din("w24", [4, P, FT, W_CHUNK], bf16)   # pre-chunked W2.T
    g1d = din("g1d", [P, FT, P], bf16)        # diag(g1) per block
    b2d = din("b2d", [P, FT, P], bf16)        # diag(b2 + be1) per block
    bqp = din("bqp", [P, 1])
    bkp = din("bkp", [P, 1])
    bvp = din("bvp", [P, 1])
    b1p = din("b1p", [P, FT])                 # b1 + W1 @ be1
    g2p = din("g2p", [P, FT])
    be2p = din("be2p", [P, FT])
    nri = din("nri", [1, 2], i32)
    nrm = din("nrm", [1, TQ], i32)
    y = nc.dram_tensor("y", [H, TQ], f32, kind="ExternalOutput").ap()

    a2a_in = nc.dram_tensor("a2a_in", [NCORES, P, TQ], bf16).ap()
    a2a_out = nc.dram_tensor("a2a_out", [NCORES, P, TQ], bf16).ap()

    with tile.TileContext(nc) as tc, ExitStack() as ctx:
        const = ctx.enter_context(tc.tile_pool(name="const", bufs=1))
        acts = ctx.enter_context(tc.tile_pool(name="acts", bufs=1))
        wpool = ctx.enter_context(tc.tile_pool(name="w", bufs=3))

        # ---- early DMAs: weights for b0 QKV, then x tiles (b0 first) ----
        xpctx = ExitStack()
        xp = xpctx.enter_context(tc.tile_pool(name="xp", bufs=1))
        wq_s = xp.tile([P, FT, P], bf16)
        nc.sync.dma_start(wq_s[:], wq4)
        wk_s = xp.tile([P, FT, P], bf16)
        nc.sync.dma_start(wk_s[:], wk4)
        xt_r = xT.rearrange("(t p) (b n) -> p b t n", p=P, b=B)
        xt = [[None] * FT for _ in range(B)]
        for b in range(B):
            for kt in range(FT):
                xt[b][kt] = xp.tile([P, S], bf16, tag=f"xt{b}_{kt}",
                                    name=f"xt{b}_{kt}")
        qs = [nc.sync, nc.scalar, nc.gpsimd]
        for kt in range(FT):
            qs[kt % 3].dma_start(xt[0][kt][:], xt_r[:, 0, kt, :])
        wv_s = xp.tile([P, FT, P], bf16)
        nc.sync.dma_start(wv_s[:], wv4)
        for kt in range(FT):
            qs[kt % 3].dma_start(xt[1][kt][:], xt_r[:, 1, kt, :])

        # constants (gpsimd queue so they don't delay x)
        bq_s = const.tile([P, 1], f32)
        nc.gpsimd.dma_start(bq_s[:], bqp)
        bk_s = const.tile([P, 1], f32)
        nc.gpsimd.dma_start(bk_s[:], bkp)
        bv_s = const.tile([P, 1], f32)
        nc.gpsimd.dma_start(bv_s[:], bvp)
        b1_s = const.tile([P, FT], f32)
        nc.gpsimd.dma_start(b1_s[:], b1p)
        g2_s = const.tile([P, FT], f32)
        nc.gpsimd.dma_start(g2_s[:], g2p)
        be2_s = const.tile([P, FT], f32)
        nc.gpsimd.dma_start(be2_s[:], be2p)
        nri_s = const.tile([1, 2], i32)
        nc.gpsimd.dma_start(nri_s[:], nri)
        nrm_s = const.tile([1, TQ], i32)
        nc.gpsimd.dma_start(nrm_s[:], nrm)
        g1d_s = const.tile([P, FT, P], bf16)
        nc.gpsimd.dma_start(g1d_s[:], g1d)
        b2d_s = const.tile([P, FT, P], bf16)
        nc.gpsimd.dma_start(b2d_s[:], b2d)
        xres_s = const.tile([P, FT, TQ], bf16)
        nc.gpsimd.dma_start(xres_s[:], xresb)
        ones_b = const.tile([P, 1], bf16)
        nc.vector.memset(ones_b[:], 1.0)
        ones_w = const.tile([P, TQ], bf16)
        nc.vector.memset(ones_w[:], 1.0)
        identb = const.tile([P, P], bf16)
        make_identity(nc, identb)
        eps_s = const.tile([P, 1], f32)
        nc.vector.memset(eps_s[:], EPS)
        warm_s = const.tile([P, 1], f32)
        nc.scalar.activation(warm_s[:], eps_s[:], AF.Exp)

        # persistent activations
        qt_s = acts.tile([P, B, S], bf16)
        kt_s = acts.tile([P, B, S], bf16)
        v_s = acts.tile([P, B, KB, 2, HD + 1], bf16)
        ctxF_s = acts.tile([P, FT, TQ], bf16)
        ln1b_s = acts.tile([P, FT, TQ], bf16)   # LN1 "z" (gamma folded into W1)
        hT_s = acts.tile([P, FT, TQ], bf16)
        for b in range(B):
            nc.vector.memset(v_s[:, b, :, :, HD:HD + 1], 1.0)
        vtctx = ExitStack()
        vtp = vtctx.enter_context(tc.tile_pool(name="vtp", bufs=1))
        vt0 = vtp.tile([P, S], bf16)            # V^T staging per batch
        vt1 = vtp.tile([P, S], bf16)

        # ---------------- batch-0 Q/K (kt-outer for early start) ----------
        with tc.tile_pool(name="psQK", bufs=1, space="PSUM") as psQK:
            pq = psQK.tile([P, QB, TQ], f32, tag="pq")
            pk = psQK.tile([P, QB, TQ], f32, tag="pk")
            for kt in range(FT):
                for c in range(QB):
                    nc.tensor.matmul(pq[:, c, :], wq_s[:, kt, :],
                                     xt[0][kt][:, ts(c, TQ)],
                                     start=(kt == 0), stop=(kt == FT - 1))
                for c in range(QB):
                    nc.tensor.matmul(pk[:, c, :], wk_s[:, kt, :],
                                     xt[0][kt][:, ts(c, TQ)],
                                     start=(kt == 0), stop=(kt == FT - 1))
            for c in range(QB):
                nc.scalar.activation(qt_s[:, 0, ts(c, TQ)], pq[:, c, :],
                                     AF.Identity, bias=bq_s[:, 0:1])
                nc.scalar.activation(kt_s[:, 0, ts(c, TQ)], pk[:, c, :],
                                     AF.Identity, bias=bk_s[:, 0:1])

        # batch-0 V: V^T chunks then PE transposes into v_s
        with tc.tile_pool(name="psV0", bufs=2, space="PSUM") as psV0, \
             tc.tile_pool(name="psT0", bufs=2, space="PSUM") as psT0:
            for c in range(QB):
                ps = psV0.tile([P, TQ], f32, tag="pv")
                for kt in range(FT):
                    nc.tensor.matmul(ps[:], wv_s[:, kt, :],
                                     xt[0][kt][:, ts(c, TQ)],
                                     start=(kt == 0), stop=(kt == FT - 1))
                nc.scalar.activation(vt0[:, ts(c, TQ)], ps[:],
                                     AF.Identity, bias=bv_s[:, 0:1])
            for kb in range(KB):
                pt = psT0.tile([P, P], f32, tag="pt")
                nc.tensor.matmul(pt[:], vt0[:, ts(kb, P)], identb[:],
                                 start=True, stop=True)
                nc.vector.tensor_copy(
                    v_s[:, 0, kb, :, 0:HD],
                    pt.rearrange("p (h d) -> p h d", h=2))

        # ---------- batch-1 QKV (serial, before attention) ----------
        with tc.tile_pool(name="psB1", bufs=2, space="PSUM") as psB1, \
             tc.tile_pool(name="psT1", bufs=2, space="PSUM") as psT1:
            for w_s, dst, bias in [(wq_s, qt_s, bq_s), (wk_s, kt_s, bk_s)]:
                for c in range(QB):
                    ps = psB1.tile([P, TQ], f32, tag="pb1", name="pb1")
                    for kt in range(FT):
                        nc.tensor.matmul(ps[:], w_s[:, kt, :],
                                         xt[1][kt][:, ts(c, TQ)],
                                         start=(kt == 0), stop=(kt == FT - 1))
                    nc.scalar.activation(dst[:, 1, ts(c, TQ)], ps[:],
                                         AF.Identity, bias=bias[:, 0:1])
            for c in range(QB):
                ps = psB1.tile([P, TQ], f32, tag="pb1", name="pb1")
                for kt in range(FT):
                    nc.tensor.matmul(ps[:], wv_s[:, kt, :],
                                     xt[1][kt][:, ts(c, TQ)],
                                     start=(kt == 0), stop=(kt == FT - 1))
                nc.scalar.activation(vt1[:, ts(c, TQ)], ps[:],
                                     AF.Identity, bias=bv_s[:, 0:1])
            for kb in range(KB):
                pt = psT1.tile([P, P], f32, tag="pt1", name="pt1")
                nc.tensor.matmul(pt[:], vt1[:, ts(kb, P)], identb[:],
                                 start=True, stop=True)
                nc.vector.tensor_copy(
                    v_s[:, 1, kb, :, 0:HD],
                    pt.rearrange("p (h d) -> p h d", h=2))

        vtctx.close()

        # ---------------- attention ----------------
        with tc.tile_pool(name="psS", bufs=2, space="PSUM") as psS, \
             tc.tile_pool(name="psC", bufs=2, space="PSUM") as psC, \
             tc.tile_pool(name="esb", bufs=2) as esb, \
             tc.tile_pool(name="rec", bufs=2) as rec:

            def attn_unit(b, qb, do_drain):
                ps_c0 = psC.tile([P, TQ], f32, tag="c0")
                ps_c1 = psC.tile([P, TQ], f32, tag="c1")
                for kb in range(KB):
                    last = (kb == KB - 1)
                    ps = psS.tile([P, 2, TQ], f32, tag="s")
                    nc.tensor.matmul(ps[:, 0, :], kt_s[0:HD, b, ts(kb, P)],
                                     qt_s[0:HD, b, ts(qb, TQ)],
                                     start=True, stop=True)
                    nc.tensor.matmul(ps[:, 1, :], kt_s[HD:P, b, ts(kb, P)],
                                     qt_s[HD:P, b, ts(qb, TQ)],
                                     start=True, stop=True)
                    e = esb.tile([P, 2, TQ], bf16, tag="e")
                    if kb in DVE_KB:
                        # Schraudolph exp on DVE: i32(A*x+B) bits read as f32
                        fv = esb.tile([P, 2, TQ], f32, tag="fv", bufs=1)
                        nc.vector.tensor_scalar(
                            out=fv.bitcast(i32)[:], in0=ps[:], scalar1=SCH_A,
                            scalar2=SCH_B, op0=ALU.mult, op1=ALU.add)
                        nc.vector.tensor_copy(e[:], fv[:])
                    else:
                        nc.scalar.activation(e[:], ps[:], AF.Exp)
                    nc.tensor.matmul(ps_c0[0:HD + 1, :], v_s[:, b, kb, 0, :],
                                     e[:, 0, :], start=(kb == 0), stop=last)
                    nc.tensor.matmul(ps_c1[0:HD + 1, :], v_s[:, b, kb, 1, :],
                                     e[:, 1, :], start=(kb == 0), stop=last)
                # normalize rows 0-63 by row 64; ship straight to a2a_in
                j = b * QB + qb
                for h, psc in ((0, ps_c0), (1, ps_c1)):
                    sr = rec.tile([HD + 1, TQ], f32, tag=f"sr{h}",
                                  name=f"sr{h}")
                    nc.vector.tensor_copy(sr[HD:HD + 1, :], psc[HD:HD + 1, :])
                    rr = rec.tile([1, TQ], f32, tag=f"rr{h}", name=f"rr{h}")
                    nc.gpsimd.dma_start(rr[:], sr[HD:HD + 1, :])
                    nc.vector.reciprocal_approx_fast(rr[:], rr[:])
                    rb = rec.tile([HD, TQ], f32, tag=f"rb{h}", name=f"rb{h}")
                    nc.gpsimd.partition_broadcast(rb[:], rr[:])
                    ct = rec.tile([HD, TQ], bf16, tag=f"ct{h}", name=f"ct{h}")
                    nc.vector.tensor_tensor(out=ct[:], in0=psc[0:HD, :],
                                            in1=rb[:], op=ALU.mult)
                    nc.sync.dma_start(a2a_in[j, ts(h, HD), :], ct[:])

            for qb in range(QB):
                attn_unit(0, qb, False)
            for qb in range(QB):
                attn_unit(1, qb, False)
        xpctx.close()

        # ---------------- AllToAll ----------------
        w_pre = wpool.tile([P, FT, W_CHUNK], bf16, tag="w")
        nc.sync.dma_start(w_pre[:], wo4[0])
        nc.gpsimd.collective_compute(
            "AllToAll", ALU.bypass, replica_groups=RG8,
            ins=[a2a_in], outs=[a2a_out])
        nc.sync.dma_start(ctxF_s[:], a2a_out.rearrange("r p n -> p r n"))

        # ---------------- projection helper (feature-major out) -----------
        def proj_T(w4ap, kxmT_s, evict, psA, w_first=None, extra_mm=None):
            for half in range(H // W_CHUNK):
                if half == 0 and w_first is not None:
                    w_s = w_first
                else:
                    w_s = wpool.tile([P, FT, W_CHUNK], bf16, tag="w",
                                     name="w_s")
                    nc.sync.dma_start(w_s[:], w4ap[half])
                for mi in range(W_CHUNK // P):
                    mt_i = half * (W_CHUNK // P) + mi
                    ps = psA.tile([P, TQ], f32, tag="psA", name="psA")
                    for kt in range(FT):
                        nc.tensor.matmul(ps[:], w_s[:, kt, ts(mi, P)],
                                         kxmT_s[:, kt, :],
                                         start=(kt == 0),
                                         stop=(kt == FT - 1 and
                                               extra_mm is None))
                    if extra_mm is not None:
                        extra_mm(mt_i, ps)
                    evict(mt_i, ps)

        # ------------- stats accumulators (delayed emission) --------------
        def make_stats(lnp, psSt, name):
            st0 = psSt.tile([1, TQ], f32, tag=f"{name}0", name=f"{name}0")
            st1 = psSt.tile([1, TQ], f32, tag=f"{name}1", name=f"{name}1")
            pend = []

            def emit_one():
                mt_j, tbj, sqj = pend.pop(0)
                nc.tensor.matmul(st0[:], ones_b[:], tbj,
                                 start=(mt_j == 0), stop=(mt_j == FT - 1))
                nc.tensor.matmul(st1[:], ones_b[:], sqj[:],
                                 start=(mt_j == 0), stop=(mt_j == FT - 1))

            def feed(mt_i, tb):
                sq = lnp.tile([P, TQ], bf16, tag=f"sq{mt_i % 3}",
                              name="sq")
                nc.vector.tensor_tensor(out=sq[:], in0=tb, in1=tb,
                                        op=ALU.mult)
                pend.append((mt_i, tb, sq))
                if len(pend) > 1:
                    emit_one()

            def flush():
                while pend:
                    emit_one()

            return st0, st1, feed, flush

        # LN scalar chain: mu/rstd from st0/st1, broadcast to [P, TQ]
        def ln_chain(st0, st1, lnp):
            mu = lnp.tile([1, TQ], f32, tag="mu", name="mu")
            nc.vector.tensor_scalar(out=mu[:], in0=st0[:], scalar1=1.0 / H,
                                    scalar2=None, op0=ALU.mult)
            muB = lnp.tile([P, TQ], f32, tag="muB", name="muB")
            nc.gpsimd.partition_broadcast(muB[:], mu[:])
            ve = lnp.tile([1, TQ], f32, tag="ve", name="ve")
            nc.vector.tensor_scalar(out=ve[:], in0=st1[:], scalar1=1.0 / H,
                                    scalar2=None, op0=ALU.mult)
            mu2 = lnp.tile([1, TQ], f32, tag="mu2", name="mu2")
            nc.vector.tensor_tensor(out=mu2[:], in0=mu[:], in1=mu[:],
                                    op=ALU.mult)
            nc.vector.tensor_tensor(out=ve[:], in0=ve[:], in1=mu2[:],
                                    op=ALU.subtract)
            nc.vector.tensor_scalar(out=ve[:], in0=ve[:], scalar1=EPS,
                                    scalar2=None, op0=ALU.add)
            it = lnp.tile([1, TQ], i32, tag="it", name="it")
            nc.vector.tensor_scalar(out=it[:], in0=ve.bitcast(i32),
                                    scalar1=nri_s[:, 0:1], scalar2=None,
                                    op0=ALU.logical_shift_right)
            nc.vector.tensor_scalar(out=it[:], in0=it[:],
                                    scalar1=nri_s[:, 1:2],
                                    scalar2=None, op0=ALU.bitwise_xor)
            nc.vector.tensor_tensor(out=it[:], in0=it[:], in1=nrm_s[:],
                                    op=ALU.add)
            rstd = it.bitcast(f32)
            nrt = lnp.tile([1, TQ], f32, tag="nrt", name="nrt")
            for _ in range(2):
                nc.vector.tensor_tensor(out=nrt[:], in0=rstd, in1=rstd,
                                        op=ALU.mult)
                nc.vector.tensor_tensor(out=nrt[:], in0=nrt[:], in1=ve[:],
                                        op=ALU.mult)
                nc.vector.tensor_scalar(out=nrt[:], in0=nrt[:], scalar1=-0.5,
                                        scalar2=1.5, op0=ALU.mult,
                                        op1=ALU.add)
                nc.vector.tensor_tensor(out=rstd, in0=rstd, in1=nrt[:],
                                        op=ALU.mult)
            rsB = lnp.tile([P, TQ], f32, tag="rsB", name="rsB")
            nc.gpsimd.partition_broadcast(rsB[:], rstd)
            return muB, rsB

        # ---------------- phases C-F ----------------
        with tc.tile_pool(name="lnp", bufs=2) as lnp, \
             tc.tile_pool(name="psA2", bufs=3, space="PSUM") as psA2, \
             tc.tile_pool(name="psSt", bufs=1, space="PSUM") as psSt:
            t1_s = acts.tile([P, FT, TQ], f32, tag="tres")
            t1b_s = acts.tile([P, FT, TQ], bf16, tag="tbb")
            st0a, st1a, feed_a, flush_a = make_stats(lnp, psSt, "sa")

            def wo_extra(mt_i, ps):    # += x residual via identity matmul
                nc.tensor.matmul(ps[:], identb[:], xres_s[:, mt_i, :],
                                 start=False, stop=True)

            def wo_evict(mt_i, ps):
                nc.scalar.copy(t1_s[:, mt_i, :], ps[:])
                nc.scalar.copy(t1b_s[:, mt_i, :], ps[:])
                feed_a(mt_i, t1b_s[:, mt_i, :])

            proj_T(wo4, ctxF_s, wo_evict, psA2, w_first=w_pre,
                   extra_mm=wo_extra)
            flush_a()
            muB1, rsB1 = ln_chain(st0a, st1a, lnp)
            for ft in range(FT):       # ln1b = (t1 - mu) * rstd  (bf16)
                z = lnp.tile([P, TQ], f32, tag=f"z{ft % 2}", name="z")
                nc.vector.tensor_tensor(out=z[:], in0=t1_s[:, ft, :],
                                        in1=muB1[:], op=ALU.subtract)
                nc.vector.tensor_tensor(out=ln1b_s[:, ft, :], in0=z[:],
                                        in1=rsB1[:], op=ALU.mult)

            def relu_evict(mt_i, ps):  # relu(ps + b1') on ACT
                nc.scalar.activation(hT_s[:, mt_i, :], ps[:], AF.Relu,
                                     bias=b1_s[:, mt_i:mt_i + 1])

            proj_T(w14, ln1b_s, relu_evict, psA2)

            t2_s = acts.tile([P, FT, TQ], f32, tag="tres")
            t2b_s = acts.tile([P, FT, TQ], bf16, tag="tbb")
            st0b, st1b, feed_b, flush_b = make_stats(lnp, psSt, "sb")

            def w2_extra(mt_i, ps):    # += g1*z (residual) and b2+be1
                nc.tensor.matmul(ps[:], g1d_s[:, mt_i, :],
                                 ln1b_s[:, mt_i, :], start=False, stop=False)
                nc.tensor.matmul(ps[:], b2d_s[:, mt_i, :], ones_w[:],
                                 start=False, stop=True)

            def w2_evict(mt_i, ps):
                nc.scalar.copy(t2_s[:, mt_i, :], ps[:])
                nc.scalar.copy(t2b_s[:, mt_i, :], ps[:])
                feed_b(mt_i, t2b_s[:, mt_i, :])

            proj_T(w24, hT_s, w2_evict, psA2, extra_mm=w2_extra)
            flush_b()
            muB2, rsB2 = ln_chain(st0b, st1b, lnp)
            y_r = y.rearrange("(t p) n -> p t n", p=P)
            for ft in range(FT):
                z = lnp.tile([P, TQ], f32, tag=f"z{ft % 2}", name="z")
                nc.vector.tensor_tensor(out=z[:], in0=t2_s[:, ft, :],
                                        in1=muB2[:], op=ALU.subtract)
                zg = lnp.tile([P, TQ], f32, tag=f"zg{ft % 2}", name="zg")
                nc.vector.scalar_tensor_tensor(
                    out=zg[:], in0=z[:], scalar=g2_s[:, ft:ft + 1],
                    in1=rsB2[:], op0=ALU.mult, op1=ALU.mult)
                yv = lnp.tile([P, TQ], f32, tag=f"yv{ft % 2}", name="yv")
                nc.scalar.activation(yv[:], zg[:], AF.Identity,
                                     bias=be2_s[:, ft:ft + 1])
                nc.sync.dma_start(y_r[:, ft, :], yv[:])

    nc.compile()
    return nc


_NC_CACHE = {}


def _get_nc():
    if 'nc' not in _NC_CACHE:
        _NC_CACHE['nc'] = build_kernel()
    return _NC_CACHE['nc']


def _bf(a):
    return np.ascontiguousarray(np.asarray(a, np.float32)).astype(
        ml_dtypes.bfloat16)


def _tile_w(wcolslice):
    # [H, 128] -> [P, FT, P] (kt-partition-tiled, contiguous)
    return np.ascontiguousarray(
        wcolslice.reshape(FT, P, P).transpose(1, 0, 2))


def _chunk_w(wT):
    # [H, H] -> [4, P, FT, W_CHUNK]
    w = wT.reshape(FT, P, 4, W_CHUNK)
    return np.ascontiguousarray(w.transpose(2, 1, 0, 3))


def make_in_maps(x, Wq, bq, Wk, bk, Wv, bv, Wo, bo, W1, b1, W2, b2,
                 g1, be1, g2, be2):
    def pt(v):
        return np.ascontiguousarray(np.asarray(v, np.float32).reshape(FT, P).T)

    def diagt(v):  # [H] -> [P, FT, P] block diagonals
        out = np.zeros((P, FT, P), np.float32)
        vv = np.asarray(v, np.float32).reshape(FT, P)
        for ftb in range(FT):
            np.fill_diagonal(out[:, ftb, :], 0)
            out[np.arange(P), ftb, np.arange(P)] = vv[ftb]
        return out

    scale = np.float32(1.0 / np.sqrt(HD))
    x = np.asarray(x, np.float32)
    g1 = np.asarray(g1, np.float32)
    be1 = np.asarray(be1, np.float32)
    W1 = np.asarray(W1, np.float32)
    b1 = np.asarray(b1, np.float32)
    b2 = np.asarray(b2, np.float32)
    bo = np.asarray(bo, np.float32)
    xTf = np.ascontiguousarray(x.transpose(2, 0, 1).reshape(H, B * S))
    wqT = np.asarray(Wq, np.float32).T
    wkT = np.asarray(Wk, np.float32).T * scale
    wvT = np.asarray(Wv, np.float32).T
    W1g = W1 * g1[None, :]
    b1f = b1 + W1 @ be1
    b2f = b2 + be1
    shared = {
        "xT": _bf(xTf),
        "wo4": _bf(_chunk_w(np.asarray(Wo, np.float32).T)),
        "w14": _bf(_chunk_w(W1g.T)),
        "w24": _bf(_chunk_w(np.asarray(W2, np.float32).T)),
        "g1d": _bf(diagt(g1)),
        "b2d": _bf(diagt(b2f)),
        "b1p": pt(b1f),
        "g2p": pt(g2), "be2p": pt(be2),
        "nri": np.array([[1, -1]], np.int32),
        "nrm": np.full((1, TQ), MAGIC, np.int32),
    }
    in_maps = []
    for c in range(NCORES):
        hs = slice(P * c, P * (c + 1))
        b, sl = c // 4, (c % 4) * TQ
        m = dict(shared)
        m["wq4"] = _bf(_tile_w(wqT[:, hs]))
        m["wk4"] = _bf(_tile_w(wkT[:, hs]))
        m["wv4"] = _bf(_tile_w(wvT[:, hs]))
        m["bqp"] = np.ascontiguousarray(
            np.asarray(bq, np.float32)[hs].reshape(P, 1))
        m["bkp"] = np.ascontiguousarray(
            (np.asarray(bk, np.float32) * scale)[hs].reshape(P, 1))
        m["bvp"] = np.ascontiguousarray(
            np.asarray(bv, np.float32)[hs].reshape(P, 1))
        xslice = (x[b, sl:sl + TQ, :].T + bo[:, None])  # [H, TQ] + bo
        m["xresb"] = _bf(xslice.reshape(FT, P, TQ).transpose(1, 0, 2))
        in_maps.append(m)
    return in_maps


def kernel(x, Wq, bq, Wk, bk, Wv, bv, Wo, bo, W1, b1, W2, b2,
           g1, be1, g2, be2):
    x = np.asarray(x)
    nc = _get_nc()
    in_maps = make_in_maps(x, Wq, bq, Wk, bk, Wv, bv, Wo, bo,
                           W1, b1, W2, b2, g1, be1, g2, be2)
    res = run_bass_kernel_spmd(nc, in_maps, list(range(NCORES)))
    out = np.empty((B, S, H), np.float32)
    for c in range(NCORES):
        b, sl = c // 4, (c % 4) * TQ
        out[b, sl:sl + TQ, :] = np.asarray(res.results[c]["y"]).T
    return out
